# revision 5
# baseline (speedup 1.0000x reference)
"""Group MoE layer (2 groups x 4 experts, top-1 group / top-2 expert routing)
on 8 Trainium2 NeuronCores via expert parallelism.

Strategy:
  - Host computes the (tiny) routing: language-gate argmax over groups,
    per-group expert top-k + softmax weights.
  - Tokens are dispatched by (group, expert) assignment: core c = g*4+e
    receives exactly the tokens routed to expert (g, e), padded to a common
    capacity C (SPMD: all cores run the same program).
  - Each core runs the dense FFN for its expert:
        Y^T = W2 @ relu(W1 @ X^T + b1) + b2      (tokens in the moving dim)
    with bf16 weights/activations and fp32 PSUM accumulation.
  - Host scatter-adds the weighted expert outputs back into the full output.
"""

import numpy as np
import ml_dtypes

import concourse.bacc as bacc
import concourse.mybir as mybir
from concourse import tile
from concourse import bass_utils

B, L, D, H = 2, 2048, 1024, 4096
G, E = 2, 4
NCORES = G * E
PART = 128
TOK_BLK = 512

_BF16 = ml_dtypes.bfloat16

_program_cache: dict[tuple, object] = {}


def _build(C: int, d: int = D, h: int = H, tok_blk: int = TOK_BLK):
    """Build + compile the per-core expert FFN program for capacity C."""
    key = (C, d, h, tok_blk)
    if key in _program_cache:
        return _program_cache[key]

    nd = d // PART
    nh = h // PART
    nblk = (C + tok_blk - 1) // tok_blk

    bf16 = mybir.dt.bfloat16
    f32 = mybir.dt.float32

    nc = bacc.Bacc("TRN2", target_bir_lowering=False, debug=False,
                   num_devices=NCORES)

    xt = nc.dram_tensor("xt", [d, C], bf16, kind="ExternalInput")
    w1t = nc.dram_tensor("w1t", [d, h], bf16, kind="ExternalInput")
    w2t = nc.dram_tensor("w2t", [h, d], bf16, kind="ExternalInput")
    b1t = nc.dram_tensor("b1t", [PART, nh], f32, kind="ExternalInput")
    b2t = nc.dram_tensor("b2t", [PART, nd], f32, kind="ExternalInput")
    yt = nc.dram_tensor("yt", [d, C], f32, kind="ExternalOutput")

    with tile.TileContext(nc) as tc:
        with (
            tc.tile_pool(name="wpool", bufs=1) as wpool,
            tc.tile_pool(name="h1pool", bufs=nh) as h1pool,
            tc.tile_pool(name="ypool", bufs=nd) as ypool,
            tc.tile_pool(name="ps1", bufs=4, space="PSUM") as ps1,
            tc.tile_pool(name="ps2", bufs=4, space="PSUM") as ps2,
        ):
            # Chunked loads ordered so the PE can start after ~2MB arrives:
            # x block 0, then W1 h-chunk 0, then the rest in need-order.
            HCH = 512                      # W1 h-chunk width (4 h-tiles)
            nhc = h // HCH
            x_sb = [[None] * nblk for _ in range(nd)]
            w1_sb = [[None] * nhc for _ in range(nd)]

            def load_x(blk):
                n0 = blk * tok_blk
                n = min(tok_blk, C - n0)
                for di in range(nd):
                    t = wpool.tile([PART, n], bf16, tag=f"x_{di}_{blk}")
                    nc.sync.dma_start(
                        out=t[:, :],
                        in_=xt.ap()[di * PART:(di + 1) * PART, n0:n0 + n])
                    x_sb[di][blk] = t

            def load_w1(hc):
                for di in range(nd):
                    t = wpool.tile([PART, HCH], bf16, tag=f"w1_{di}_{hc}")
                    nc.sync.dma_start(
                        out=t[:, :],
                        in_=w1t.ap()[di * PART:(di + 1) * PART,
                                     hc * HCH:(hc + 1) * HCH])
                    w1_sb[di][hc] = t

            load_x(0)
            load_w1(0)
            for hc in range(1, nhc):
                load_w1(hc)
            for blk in range(1, nblk):
                load_x(blk)
            w2_sb = []
            for hi in range(nh):
                t = wpool.tile([PART, d], bf16, tag=f"w2_{hi}")
                nc.sync.dma_start(out=t[:, :],
                                  in_=w2t.ap()[hi * PART:(hi + 1) * PART, :])
                w2_sb.append(t)
            b1_sb = wpool.tile([PART, nh], f32, tag="b1")
            nc.sync.dma_start(out=b1_sb[:, :], in_=b1t.ap()[:, :])
            b2_sb = wpool.tile([PART, nd], f32, tag="b2")
            nc.sync.dma_start(out=b2_sb[:, :], in_=b2t.ap()[:, :])

            for blk in range(nblk):
                n0 = blk * tok_blk
                n = min(tok_blk, C - n0)
                h1_tiles = []
                for hi in range(nh):
                    hc, ho = divmod(hi * PART, HCH)
                    ps = ps1.tile([PART, tok_blk], f32, tag="ps1")
                    for di in range(nd):
                        nc.tensor.matmul(
                            ps[:, :n],
                            w1_sb[di][hc][:, ho:ho + PART],
                            x_sb[di][blk][:, :n],
                            start=(di == 0), stop=(di == nd - 1),
                        )
                    h1 = h1pool.tile([PART, tok_blk], bf16, tag="h1")
                    nc.scalar.activation(h1[:, :n], ps[:, :n],
                                         mybir.ActivationFunctionType.Relu,
                                         bias=b1_sb[:, hi:hi + 1], scale=1.0)
                    h1_tiles.append(h1)
                for di in range(nd):
                    ps = ps2.tile([PART, tok_blk], f32, tag="ps2")
                    for hi in range(nh):
                        nc.tensor.matmul(
                            ps[:, :n],
                            w2_sb[hi][:, di * PART:(di + 1) * PART],
                            h1_tiles[hi][:, :n],
                            start=(hi == 0), stop=(hi == nh - 1),
                        )
                    y = ypool.tile([PART, tok_blk], f32, tag="y")
                    nc.vector.tensor_scalar_add(y[:, :n], ps[:, :n],
                                                b2_sb[:, di:di + 1])
                    nc.sync.dma_start(
                        out=yt.ap()[di * PART:(di + 1) * PART, n0:n0 + n],
                        in_=y[:, :n])

    nc.compile()
    _program_cache[key] = nc
    return nc


def _route(x, bn, Wlg, blg, Wg, k):
    """Numpy replica of the reference routing. Returns per-(g,e) assignment."""
    glog = bn @ Wlg.T + blg                       # (N, G)
    sel_group = np.argmax(glog, axis=1)           # (N,)
    assign = []
    for g in range(Wg.shape[0]):
        logits = x @ Wg[g].T                      # (N, E)
        order = np.argsort(-logits, axis=1, kind="stable")
        sel = order[:, :k]                        # (N, k)
        top = np.take_along_axis(logits, sel, axis=1).astype(np.float32)
        m = top.max(axis=1, keepdims=True)
        ex = np.exp(top - m)
        w = ex / ex.sum(axis=1, keepdims=True)    # (N, k)
        assign.append((sel, w))
    return sel_group, assign


def kernel(**inputs) -> np.ndarray:
    xs = np.asarray(inputs["xs"], np.float32)
    bn = np.asarray(inputs["bottle_neck"], np.float32)
    Wlg = np.asarray(inputs["Wlg"], np.float32)
    blg = np.asarray(inputs["blg"], np.float32)
    Wg = np.asarray(inputs["Wg"], np.float32)
    W1 = np.asarray(inputs["W1"], np.float32)
    b1 = np.asarray(inputs["b1"], np.float32)
    W2 = np.asarray(inputs["W2"], np.float32)
    b2 = np.asarray(inputs["b2"], np.float32)
    k = int(np.asarray(inputs["top_k"]))

    Bx, Lx, d = xs.shape
    hdim = W1.shape[2]
    N = Bx * Lx
    x = xs.reshape(N, d)
    bnf = bn.reshape(N, d)

    sel_group, assign = _route(x, bnf, Wlg, blg, Wg, k)

    # Token sets per (group, expert) core.
    idxs, wgts = [], []
    for c in range(NCORES):
        g, e = divmod(c, E)
        sel, w = assign[g]
        mask = (sel_group == g)[:, None] & (sel == e)
        rows, cols = np.nonzero(mask)
        idxs.append(rows)
        wgts.append(w[rows, cols])

    cnt_max = max(len(i) for i in idxs)
    C = max(PART, -(-cnt_max // PART) * PART)     # pad capacity to 128

    nc = _build(C, d, hdim)

    in_maps = []
    for c in range(NCORES):
        g, e = divmod(c, E)
        xt = np.zeros((d, C), _BF16)
        cnt = len(idxs[c])
        if cnt:
            xt[:, :cnt] = x[idxs[c]].T.astype(_BF16)
        in_maps.append({
            "xt": xt,
            "w1t": np.ascontiguousarray(W1[g, e].T).astype(_BF16),
            "w2t": np.ascontiguousarray(W2[g, e].T).astype(_BF16),
            "b1t": np.ascontiguousarray(b1[g, e].reshape(hdim // PART, PART).T),
            "b2t": np.ascontiguousarray(b2[g, e].reshape(d // PART, PART).T),
        })

    res = bass_utils.run_bass_kernel_spmd(nc, in_maps, core_ids=list(range(NCORES)))

    out = np.zeros((N, d), np.float32)
    for c in range(NCORES):
        cnt = len(idxs[c])
        if cnt == 0:
            continue
        yc = res.results[c]["yt"][:, :cnt].T      # (cnt, d)
        out[idxs[c]] += wgts[c][:, None] * yc
    return out.reshape(Bx, Lx, d).astype(np.float32)
